# revision 64
# baseline (speedup 1.0000x reference)
"""Causal (diagonal=1) multi-head-of-one attention for trn2, 8-core SPMD.

Reference computation (fp32):
    k = key @ Wk.T; q = query @ Wq.T; v = value @ Wv.T       # [B,T,H]
    qk = (q @ k.T) / sqrt(E)                                  # [B,T,T]
    qk masked with tril(ones, k=1) and padding_mask           # -inf outside
    attn = softmax(qk, -1) @ v                                # [B,T,H]

Sharding: data-parallel over batch, 2 batches per core, no collectives.

The k-projection is folded away on the host: qk = q @ k.T =
query @ (Wq.T @ Wk) @ key.T, so the device multiplies query by the
precomputed W = Wq.T @ Wk and scores directly against the raw key.
This removes 1024^3 MACs per batch (~23% of total PE work).

Device kernel (per core, per batch), all matmuls bf16 with fp32 PSUM:
    tmpT[f,t] = sum_e W[e,f].T-chunks @ queryT[e,t]           (proj)
    v[s,h]    = valueT[e,s].T @ WvT[e,h]
    sT[s,t]   = keyT-chunk.T @ tmpT  (only causally-live s-chunks)
    pT[s,t]   = exp(sT/32)  (ScalarE; max-subtraction skipped: |s/32| <~ 6)
    pT        = affine_select(pT, keep j<=i+1, else 0)        (GPSIMD)
    num[t,h]  = pT-chunk.T @ v ; den[t,1] = pT-chunk.T @ pad01
    out[t,h]  = num * reciprocal(den)                         (VectorE)

Perf notes (205us vs 183us bf16-PE floor on trn2 @2.4GHz):
  - k-proj folded into W on host (-27us/batch of PE work)
  - exact-causal 43-block scores via 512-wide column-trimmed groups
  - batch-0 q-projection runs ec-outer (two passes of 8 PSUM banks) so
    matmuls consume input chunks as their DMAs land
  - DMA issue is serialized ~0.65us/dma_start per issuing engine; weight
    loads issue from the Scalar HWDGE queue in parallel with Sync
  - junk-matmul warmup trips the HAM clock gate (1.2->2.4GHz, ~4.5us
    after first PE activity) before the first data-dependent matmul
  - bf16 output (upcast on host) halves output DMA

padding_mask is folded in exactly on the host: v rows and the denominator
column are scaled by pad01 = (padding_mask == 0), which equals softmax
with -inf at padded keys.
"""
from contextlib import ExitStack

import numpy as np
import ml_dtypes

import concourse.bass as bass
import concourse.mybir as mybir
import concourse.tile as tile
from concourse.bass_utils import run_bass_kernel_spmd

BF16 = mybir.dt.bfloat16
F32 = mybir.dt.float32
P = 128
T = 1024           # sequence length
E = 1024           # embed dim
H = 1024           # head dim
NB = 16            # full batch
NCORES = 8
BPC = NB // NCORES  # batches per core
NC = T // P        # 128-chunks per dim (8)
SCALE = 1.0 / 32.0  # 1/sqrt(E)

_nc_cache = None


# --- walrus workaround: one sync-wait per instruction ---------------------
def _split_multi_waits(nc):
    """This walrus build rejects instructions with >1 sync wait (2 for
    EventSemaphore).  Move extra waits onto fresh same-engine NOPs placed
    immediately before the instruction; per-engine in-order execution
    preserves the gating, and semaphore updates stay on the original."""
    for fn in nc.m.functions:
        for bb in fn.blocks:
            il = bb.instructions
            idx = 0
            while idx < len(il):
                inst = il[idx]
                si = inst.sync_info
                waits = list(si.on_wait) if si and si.on_wait else []
                cap = 2 if isinstance(inst, mybir.InstEventSemaphore) else 1
                if len(waits) > cap:
                    extra, keep = waits[:-cap], waits[-cap:]
                    for j, w in enumerate(extra):
                        nop = mybir.InstNoOp(
                            name=f"I-wsplit-{inst.name}-{j}",
                            engine=inst.engine,
                            ins=[],
                            outs=[],
                            sync_info=mybir.SyncInfo(on_wait=[w], on_update=[]),
                        )
                        il.insert(idx, nop)
                        idx += 1
                    inst.sync_info = mybir.SyncInfo(
                        on_wait=keep, on_update=list(si.on_update or [])
                    )
                idx += 1


def _n_sc(ti):
    """Number of live 128-wide s-chunks for t-tile ti (cols j <= t+1)."""
    return min(ti + 2, NC)


def _emit_batch(nc, pools, b, dram):
    Exp = mybir.ActivationFunctionType.Exp
    w_q, w_v = pools["wq"], pools["wv"]
    sb, ps = pools["sb"], pools["ps"]

    # -- load inputs + projections, ordered so the first projection's DMAs
    #    issue first and later tensors stream in behind the PE --
    # DMA issue is serialized per engine (~0.65us per dma_start DIRECT2D on
    # the issuing sequencer), so weight-chunk DMAs issue from the Scalar
    # HWDGE queue in parallel with input chunks on the Sync queue.
    def load_in(tag, dname, interleave=None, eng=None):
        # interleave: per-ec callback issuing the matching weight-chunk DMA
        # (on the other engine) right after the input chunk, so the ec-th
        # matmul's operands arrive together.
        tiles = []
        for ec in range(NC):
            if interleave is not None:
                interleave(ec)
            t = sb.tile([P, T], BF16, name=f"{tag}{ec}")
            (eng or nc.sync).dma_start(t[:], dram[dname][b, bass.ts(ec, P), :])
            tiles.append(t)
        return tiles

    qTs = [sb.tile([P, T], BF16, name=f"qTs{h}") for h in range(NC)]
    # col 1024 of each v chunk holds the pad01 column, so the softmax
    # denominator rides in the third pv matmul instead of an N=1 matmul
    v_sb = [sb.tile([P, T + 1], BF16, name=f"vsb{s}") for s in range(NC)]

    def proj_qk(w_t, x_in, x_out):
        for ht in range(NC):
            for tg in range(2):
                acc = ps.tile([P, 512], F32, name="ps")
                for ec in range(NC):
                    nc.tensor.matmul(
                        acc[:],
                        lhsT=w_t[ec][:, bass.ts(ht, P)],
                        rhs=x_in[ec][:, bass.ts(tg, 512)],
                        start=(ec == 0),
                        stop=(ec == NC - 1),
                    )
                nc.scalar.copy(x_out[ht][:, bass.ts(tg, 512)], acc[:])

    def proj_ec_outer(w_t, x_in, x_out):
        # ec-outer, two passes of 8 concurrent PSUM accumulations: each
        # ec-step consumes input chunk ec as soon as its DMA lands, so the
        # projection overlaps the initial HBM fill window instead of
        # waiting for all 16 input tiles.  Also self-warms the HAM clock.
        for half in range(2):
            accs = {}
            for ht in range(4 * half, 4 * half + 4):
                for tg in range(2):
                    accs[(ht, tg)] = ps.tile([P, 512], F32, name="ps")
            for ec in range(NC):
                for ht in range(4 * half, 4 * half + 4):
                    for tg in range(2):
                        nc.tensor.matmul(
                            accs[(ht, tg)][:],
                            lhsT=w_t[ec][:, bass.ts(ht, P)],
                            rhs=x_in[ec][:, bass.ts(tg, 512)],
                            start=(ec == 0),
                            stop=(ec == NC - 1),
                        )
            for (ht, tg), acc in accs.items():
                nc.scalar.copy(x_out[ht][:, bass.ts(tg, 512)], acc[:])

    qin = load_in("qin", "qT", interleave=pools.pop("wq_dma", None))
    if b == 0:
        proj_ec_outer(w_q, qin, qTs)
    else:
        proj_qk(w_q, qin, qTs)
    # raw key^T chunks feed the score matmuls directly (k-proj folded into W)
    kTs = load_in("kTs", "kT", interleave=pools.pop("wv_dma", None))
    vin = load_in("vin", "vT")
    padt = sb.tile([P, NC * 8], BF16, name="padt", bufs=2)
    nc.sync.dma_start(padt[:], dram["pad"][b])
    for st in range(NC):
        for hh in range(2):
            acc = ps.tile([P, 512], F32, name="ps")
            for ec in range(NC):
                nc.tensor.matmul(
                    acc[:],
                    lhsT=vin[ec][:, bass.ts(st, P)],
                    rhs=w_v[ec][:, bass.ts(hh, 512)],
                    start=(ec == 0),
                    stop=(ec == NC - 1),
                )
            nc.vector.tensor_copy(v_sb[st][:, bass.ts(hh, 512)], acc[:])
        nc.gpsimd.tensor_copy(v_sb[st][:, T:T + 1], padt[:, st * 8:st * 8 + 1])

    # -- scores^T + exp + causal zeroing --
    # 512-wide t-groups, trimmed to the causally-live column window at
    # 128-col granularity: exactly the causal-minimum 43 block-equivalents
    # in only 13 matmul groups (vs 43 at 128-wide grouping).
    GW = 512
    pT = [sb.tile([P, T], BF16, name=f"pT{s}") for s in range(NC)]
    for g in range(T // GW):
        for sc in range(min((GW * (g + 1)) // P + 1, NC)):
            off = 128 * sc - GW * g
            # live column window: t >= s - 1 first holds at t_local = off-128
            c0 = max(0, off - 128)
            cw = GW - c0
            acc = ps.tile([P, 512], F32, name="ps")
            for hc in range(NC):
                nc.tensor.matmul(
                    acc[:, :cw],
                    lhsT=kTs[hc][:, bass.ts(sc, P)],
                    rhs=qTs[hc][:, GW * g + c0: GW * g + GW],
                    start=(hc == 0),
                    stop=(hc == NC - 1),
                )
            dst = pT[sc][:, GW * g + c0: GW * g + GW]
            nc.scalar.activation(dst, acc[:, :cw], Exp, scale=SCALE)
            if off >= 0:
                # keep where t_local - s_local - (off-c0) + 1 >= 0 (j <= i+1)
                nc.gpsimd.affine_select(
                    out=dst,
                    in_=dst,
                    compare_op=mybir.AluOpType.is_ge,
                    fill=0.0,
                    base=1 - (off - c0),
                    pattern=[[1, cw]],
                    channel_multiplier=-1,
                )

    # -- attn = (pT.T @ [v | pad01]) with post-normalization --
    # three ~342-col matmuls per s-chunk (1025 moving cycles total, same
    # as 512+512+1) keep every matmul wide enough to pipeline its
    # ldweights; the denominator is column 340 of po2.
    # Ascending tile order: ending on the big ti=7 tile (3.4us of PE)
    # lets every prior tile's scale+DMA chain drain before the kernel
    # tail, which beats reordering small tiles last (two pending chains
    # at the end cost more than the ~60ns mid-phase bank-recycle stalls
    # ascending incurs at ti=2,3).
    for ti in range(NC):
        nsc = _n_sc(ti)
        po0 = ps.tile([P, 342], F32, name="ps")
        po1 = ps.tile([P, 342], F32, name="ps")
        po2 = ps.tile([P, 341], F32, name="ps")
        for sc in range(nsc):
            lhsT = pT[sc][:, bass.ts(ti, P)]
            st, sp = (sc == 0), (sc == nsc - 1)
            # po2 (carrying the denominator) first, so the reciprocal and
            # its dependent scale overlap the last two matmuls of the tile
            nc.tensor.matmul(po2[:], lhsT=lhsT, rhs=v_sb[sc][:, 684:1025],
                             start=st, stop=sp)
            nc.tensor.matmul(po0[:], lhsT=lhsT, rhs=v_sb[sc][:, 0:342],
                             start=st, stop=sp)
            nc.tensor.matmul(po1[:], lhsT=lhsT, rhs=v_sb[sc][:, 342:684],
                             start=st, stop=sp)
        r = sb.tile([P, 1], F32, name="recip", bufs=3)
        nc.vector.reciprocal(r[:], po2[:, 340:341])
        osb = sb.tile([P, T], BF16, name="osb", bufs=3)
        # one scale piece per engine (Vector/Scalar/GpSimd run in
        # parallel); out-DMA split across the Sync and Scalar HWDGE
        # queues so the two issues and transfers overlap
        nc.vector.tensor_scalar_mul(osb[:, 0:342], po0[:], r[:])
        nc.scalar.activation(osb[:, 342:684], po1[:],
                             mybir.ActivationFunctionType.Copy, scale=r[:])
        nc.vector.tensor_scalar_mul(osb[:, 684:1024], po2[:, 0:340], r[:])
        nc.sync.dma_start(dram["out"][b, bass.ts(ti, P), 0:684],
                          osb[:, 0:684])
        nc.scalar.dma_start(dram["out"][b, bass.ts(ti, P), 684:1024],
                            osb[:, 684:1024])


def _build_nc():
    nc = bass.Bass()
    dram = {
        "qT": nc.declare_dram_parameter("qT", [BPC, E, T], BF16, isOutput=False),
        "kT": nc.declare_dram_parameter("kT", [BPC, E, T], BF16, isOutput=False),
        "vT": nc.declare_dram_parameter("vT", [BPC, E, T], BF16, isOutput=False),
        # "wq" holds W = Wq.T @ Wk (k-proj folded on host)
        "wq": nc.declare_dram_parameter("wq", [E, H], BF16, isOutput=False),
        "wv": nc.declare_dram_parameter("wv", [E, H], BF16, isOutput=False),
        # pad01 pre-laid-out host-side as [P, NC*8]: col c*8+j = chunk-c
        # pad column (replicated 8x for the N=8 den matmul)
        "pad": nc.declare_dram_parameter("pad", [BPC, P, NC * 8], BF16, isOutput=False),
        "out": nc.declare_dram_parameter("out", [BPC, T, H], BF16, isOutput=True),
    }
    with tile.TileContext(nc) as tc, ExitStack() as ctx:
        sb = ctx.enter_context(tc.tile_pool(name="sb", bufs=1))
        ps = ctx.enter_context(tc.tile_pool(name="ps", bufs=8, space="PSUM"))

        pools = {"sb": sb, "ps": ps}
        for wname in ("wq", "wv"):
            pools[wname] = [
                sb.tile([P, H], BF16, name=f"{wname}{ec}") for ec in range(NC)
            ]

        def w_dma(wname, eng):
            def go(ec):
                eng.dma_start(
                    pools[wname][ec][:], dram[wname][bass.ts(ec, P), :]
                )
            return go

        # Weight DMAs interleave chunk-by-chunk with batch 0's input loads.
        # wq issues from the idle Scalar HWDGE queue at startup, in parallel
        # with qin on Sync; wv stays on Sync (Scalar is busy by then).
        pools["wq_dma"] = w_dma("wq", nc.scalar)
        pools["wv_dma"] = w_dma("wv", nc.sync)

        # Small PE warm-up starting at sequencer boot: HAM (full clock)
        # trips ~4.5us after sustained PE activity, so early junk matmuls
        # get the clock to 2.4GHz before the first DMA-fed real matmul.
        warm = sb.tile([P, 512], BF16, name="warm")
        nc.gpsimd.memset(warm[:], 0.0)
        wps = ps.tile([P, 512], F32, name="ps")
        for _ in range(11):
            nc.tensor.matmul(wps[:], lhsT=warm[:, 0:P], rhs=warm[:],
                             start=True, stop=True)

        for b in range(BPC):
            _emit_batch(nc, pools, b, dram)

    _split_multi_waits(nc)
    return nc


def _get_nc():
    global _nc_cache
    if _nc_cache is None:
        _nc_cache = _build_nc()
    return _nc_cache


def _make_in_maps(key, query, value, padding_mask, Wk, Wq, Wv):
    bf = ml_dtypes.bfloat16
    # Fold the k-projection into the q side: q @ k.T = query @ W @ key.T
    W = (Wq.astype(np.float64).T @ Wk.astype(np.float64)).astype(np.float32)
    wq = np.ascontiguousarray(W).astype(bf)  # [E, E]
    wv = np.ascontiguousarray(Wv.T).astype(bf)
    pad01 = (padding_mask.reshape(NB, T) == 0).astype(np.float32)  # [B,T]
    in_maps = []
    for c in range(NCORES):
        s = slice(BPC * c, BPC * (c + 1))
        qT = np.ascontiguousarray(query[s].transpose(0, 2, 1)).astype(bf)
        kT = np.ascontiguousarray(key[s].transpose(0, 2, 1)).astype(bf)
        vTf = value[s].transpose(0, 2, 1) * pad01[s][:, None, :]
        vT = np.ascontiguousarray(vTf).astype(bf)
        in_maps.append({
            "qT": qT, "kT": kT, "vT": vT,
            "wq": wq, "wv": wv,
            "pad": np.ascontiguousarray(
                np.repeat(
                    pad01[s].reshape(BPC, NC, P).transpose(0, 2, 1)[..., None],
                    8, axis=3,
                ).reshape(BPC, P, NC * 8)
            ).astype(bf),
        })
    return in_maps


def run_on_cores(in_maps, trace=False, **kw):
    nc = _get_nc()
    return run_bass_kernel_spmd(nc, in_maps, list(range(NCORES)), trace=trace, **kw)


def kernel(key, query, value, padding_mask, Wk, Wq, Wv):
    key = np.asarray(key)
    query = np.asarray(query)
    value = np.asarray(value)
    padding_mask = np.asarray(padding_mask)
    in_maps = _make_in_maps(key, query, value, padding_mask,
                            np.asarray(Wk), np.asarray(Wq), np.asarray(Wv))
    res = run_on_cores(in_maps)
    out = np.empty((NB, T, H), np.float32)
    for c in range(NCORES):
        out[BPC * c: BPC * (c + 1)] = res.results[c]["out"].astype(np.float32)
    return out



# revision 67
# speedup vs baseline: 1.0007x; 1.0007x over previous
"""Causal (diagonal=1) multi-head-of-one attention for trn2, 8-core SPMD.

Reference computation (fp32):
    k = key @ Wk.T; q = query @ Wq.T; v = value @ Wv.T       # [B,T,H]
    qk = (q @ k.T) / sqrt(E)                                  # [B,T,T]
    qk masked with tril(ones, k=1) and padding_mask           # -inf outside
    attn = softmax(qk, -1) @ v                                # [B,T,H]

Sharding: data-parallel over batch, 2 batches per core, no collectives.

The k-projection is folded away on the host: qk = q @ k.T =
query @ (Wq.T @ Wk) @ key.T, so the device multiplies query by the
precomputed W = Wq.T @ Wk and scores directly against the raw key.
This removes 1024^3 MACs per batch (~23% of total PE work).

Device kernel (per core, per batch), all matmuls bf16 with fp32 PSUM:
    tmpT[f,t] = sum_e W[e,f].T-chunks @ queryT[e,t]           (proj)
    v[s,h]    = valueT[e,s].T @ WvT[e,h]
    sT[s,t]   = keyT-chunk.T @ tmpT  (only causally-live s-chunks)
    pT[s,t]   = exp(sT/32)  (ScalarE; max-subtraction skipped: |s/32| <~ 6)
    pT        = affine_select(pT, keep j<=i+1, else 0)        (GPSIMD)
    num[t,h]  = pT-chunk.T @ v ; den[t,1] = pT-chunk.T @ pad01
    out[t,h]  = num * reciprocal(den)                         (VectorE)

Perf notes (205us vs 183us bf16-PE floor on trn2 @2.4GHz):
  - k-proj folded into W on host (-27us/batch of PE work)
  - exact-causal 43-block scores via 512-wide column-trimmed groups
  - batch-0 q-projection runs ec-outer (two passes of 8 PSUM banks) so
    matmuls consume input chunks as their DMAs land
  - DMA issue is serialized ~0.65us/dma_start per issuing engine; weight
    loads issue from the Scalar HWDGE queue in parallel with Sync
  - junk-matmul warmup trips the HAM clock gate (1.2->2.4GHz, ~4.5us
    after first PE activity) before the first data-dependent matmul
  - bf16 output (upcast on host) halves output DMA

padding_mask is folded in exactly on the host: v rows and the denominator
column are scaled by pad01 = (padding_mask == 0), which equals softmax
with -inf at padded keys.
"""
from contextlib import ExitStack

import numpy as np
import ml_dtypes

import concourse.bass as bass
import concourse.mybir as mybir
import concourse.tile as tile
from concourse.bass_utils import run_bass_kernel_spmd

BF16 = mybir.dt.bfloat16
F32 = mybir.dt.float32
P = 128
T = 1024           # sequence length
E = 1024           # embed dim
H = 1024           # head dim
NB = 16            # full batch
NCORES = 8
BPC = NB // NCORES  # batches per core
NC = T // P        # 128-chunks per dim (8)
SCALE = 1.0 / 32.0  # 1/sqrt(E)

_nc_cache = None


# --- walrus workaround: one sync-wait per instruction ---------------------
def _split_multi_waits(nc):
    """This walrus build rejects instructions with >1 sync wait (2 for
    EventSemaphore).  Move extra waits onto fresh same-engine NOPs placed
    immediately before the instruction; per-engine in-order execution
    preserves the gating, and semaphore updates stay on the original."""
    for fn in nc.m.functions:
        for bb in fn.blocks:
            il = bb.instructions
            idx = 0
            while idx < len(il):
                inst = il[idx]
                si = inst.sync_info
                waits = list(si.on_wait) if si and si.on_wait else []
                cap = 2 if isinstance(inst, mybir.InstEventSemaphore) else 1
                if len(waits) > cap:
                    extra, keep = waits[:-cap], waits[-cap:]
                    for j, w in enumerate(extra):
                        nop = mybir.InstNoOp(
                            name=f"I-wsplit-{inst.name}-{j}",
                            engine=inst.engine,
                            ins=[],
                            outs=[],
                            sync_info=mybir.SyncInfo(on_wait=[w], on_update=[]),
                        )
                        il.insert(idx, nop)
                        idx += 1
                    inst.sync_info = mybir.SyncInfo(
                        on_wait=keep, on_update=list(si.on_update or [])
                    )
                idx += 1


def _n_sc(ti):
    """Number of live 128-wide s-chunks for t-tile ti (cols j <= t+1)."""
    return min(ti + 2, NC)


def _emit_batch(nc, pools, b, dram):
    Exp = mybir.ActivationFunctionType.Exp
    w_q, w_v = pools["wq"], pools["wv"]
    sb, ps = pools["sb"], pools["ps"]

    # -- load inputs + projections, ordered so the first projection's DMAs
    #    issue first and later tensors stream in behind the PE --
    # DMA issue is serialized per engine (~0.65us per dma_start DIRECT2D on
    # the issuing sequencer), so weight-chunk DMAs issue from the Scalar
    # HWDGE queue in parallel with input chunks on the Sync queue.
    def load_in(tag, dname, interleave=None, eng=None):
        # interleave: per-ec callback issuing the matching weight-chunk DMA
        # (on the other engine) right after the input chunk, so the ec-th
        # matmul's operands arrive together.
        tiles = []
        for ec in range(NC):
            if interleave is not None:
                interleave(ec)
            t = sb.tile([P, T], BF16, name=f"{tag}{ec}")
            e = eng or nc.sync
            if b == 0 and tag == "qin" and ec == 0:
                # chunk 0 in two halves: the first ec-outer matmuls need
                # only cols 0:512, so they unblock ~0.7us earlier
                # (sub-tile dependency tracking gates on the half-writes)
                e.dma_start(t[:, 0:512], dram[dname][b, bass.ts(ec, P), 0:512])
                e.dma_start(t[:, 512:1024],
                            dram[dname][b, bass.ts(ec, P), 512:1024])
            else:
                e.dma_start(t[:], dram[dname][b, bass.ts(ec, P), :])
            tiles.append(t)
        return tiles

    qTs = [sb.tile([P, T], BF16, name=f"qTs{h}") for h in range(NC)]
    # col 1024 of each v chunk holds the pad01 column, so the softmax
    # denominator rides in the third pv matmul instead of an N=1 matmul
    v_sb = [sb.tile([P, T + 1], BF16, name=f"vsb{s}") for s in range(NC)]

    def proj_qk(w_t, x_in, x_out):
        for ht in range(NC):
            for tg in range(2):
                acc = ps.tile([P, 512], F32, name="ps")
                for ec in range(NC):
                    nc.tensor.matmul(
                        acc[:],
                        lhsT=w_t[ec][:, bass.ts(ht, P)],
                        rhs=x_in[ec][:, bass.ts(tg, 512)],
                        start=(ec == 0),
                        stop=(ec == NC - 1),
                    )
                nc.scalar.copy(x_out[ht][:, bass.ts(tg, 512)], acc[:])

    def proj_ec_outer(w_t, x_in, x_out):
        # ec-outer, two passes of 8 concurrent PSUM accumulations: each
        # ec-step consumes input chunk ec as soon as its DMA lands, so the
        # projection overlaps the initial HBM fill window instead of
        # waiting for all 16 input tiles.  Also self-warms the HAM clock.
        for half in range(2):
            accs = {}
            for ht in range(4 * half, 4 * half + 4):
                for tg in range(2):
                    accs[(ht, tg)] = ps.tile([P, 512], F32, name="ps")
            for ec in range(NC):
                for ht in range(4 * half, 4 * half + 4):
                    for tg in range(2):
                        nc.tensor.matmul(
                            accs[(ht, tg)][:],
                            lhsT=w_t[ec][:, bass.ts(ht, P)],
                            rhs=x_in[ec][:, bass.ts(tg, 512)],
                            start=(ec == 0),
                            stop=(ec == NC - 1),
                        )
            for (ht, tg), acc in accs.items():
                nc.scalar.copy(x_out[ht][:, bass.ts(tg, 512)], acc[:])

    qin = load_in("qin", "qT", interleave=pools.pop("wq_dma", None))
    if b == 0:
        proj_ec_outer(w_q, qin, qTs)
    else:
        proj_qk(w_q, qin, qTs)
    # raw key^T chunks feed the score matmuls directly (k-proj folded into W)
    kTs = load_in("kTs", "kT", interleave=pools.pop("wv_dma", None))
    vin = load_in("vin", "vT")
    padt = sb.tile([P, NC * 8], BF16, name="padt", bufs=2)
    nc.sync.dma_start(padt[:], dram["pad"][b])
    for st in range(NC):
        for hh in range(2):
            acc = ps.tile([P, 512], F32, name="ps")
            for ec in range(NC):
                nc.tensor.matmul(
                    acc[:],
                    lhsT=vin[ec][:, bass.ts(st, P)],
                    rhs=w_v[ec][:, bass.ts(hh, 512)],
                    start=(ec == 0),
                    stop=(ec == NC - 1),
                )
            nc.vector.tensor_copy(v_sb[st][:, bass.ts(hh, 512)], acc[:])
        nc.gpsimd.tensor_copy(v_sb[st][:, T:T + 1], padt[:, st * 8:st * 8 + 1])

    # -- scores^T + exp + causal zeroing --
    # 512-wide t-groups, trimmed to the causally-live column window at
    # 128-col granularity: exactly the causal-minimum 43 block-equivalents
    # in only 13 matmul groups (vs 43 at 128-wide grouping).
    GW = 512
    pT = [sb.tile([P, T], BF16, name=f"pT{s}") for s in range(NC)]
    for g in range(T // GW):
        for sc in range(min((GW * (g + 1)) // P + 1, NC)):
            off = 128 * sc - GW * g
            # live column window: t >= s - 1 first holds at t_local = off-128
            c0 = max(0, off - 128)
            cw = GW - c0
            acc = ps.tile([P, 512], F32, name="ps")
            for hc in range(NC):
                nc.tensor.matmul(
                    acc[:, :cw],
                    lhsT=kTs[hc][:, bass.ts(sc, P)],
                    rhs=qTs[hc][:, GW * g + c0: GW * g + GW],
                    start=(hc == 0),
                    stop=(hc == NC - 1),
                )
            dst = pT[sc][:, GW * g + c0: GW * g + GW]
            nc.scalar.activation(dst, acc[:, :cw], Exp, scale=SCALE)
            if off >= 0:
                # keep where t_local - s_local - (off-c0) + 1 >= 0 (j <= i+1)
                nc.gpsimd.affine_select(
                    out=dst,
                    in_=dst,
                    compare_op=mybir.AluOpType.is_ge,
                    fill=0.0,
                    base=1 - (off - c0),
                    pattern=[[1, cw]],
                    channel_multiplier=-1,
                )

    # -- attn = (pT.T @ [v | pad01]) with post-normalization --
    # three ~342-col matmuls per s-chunk (1025 moving cycles total, same
    # as 512+512+1) keep every matmul wide enough to pipeline its
    # ldweights; the denominator is column 340 of po2.
    # Ascending tile order: ending on the big ti=7 tile (3.4us of PE)
    # lets every prior tile's scale+DMA chain drain before the kernel
    # tail, which beats reordering small tiles last (two pending chains
    # at the end cost more than the ~60ns mid-phase bank-recycle stalls
    # ascending incurs at ti=2,3).
    for ti in range(NC):
        nsc = _n_sc(ti)
        po0 = ps.tile([P, 342], F32, name="ps")
        po1 = ps.tile([P, 342], F32, name="ps")
        po2 = ps.tile([P, 341], F32, name="ps")
        for sc in range(nsc):
            lhsT = pT[sc][:, bass.ts(ti, P)]
            st, sp = (sc == 0), (sc == nsc - 1)
            # po2 (carrying the denominator) first, so the reciprocal and
            # its dependent scale overlap the last two matmuls of the tile
            nc.tensor.matmul(po2[:], lhsT=lhsT, rhs=v_sb[sc][:, 684:1025],
                             start=st, stop=sp)
            nc.tensor.matmul(po0[:], lhsT=lhsT, rhs=v_sb[sc][:, 0:342],
                             start=st, stop=sp)
            nc.tensor.matmul(po1[:], lhsT=lhsT, rhs=v_sb[sc][:, 342:684],
                             start=st, stop=sp)
        r = sb.tile([P, 1], F32, name="recip", bufs=3)
        nc.vector.reciprocal(r[:], po2[:, 340:341])
        osb = sb.tile([P, T], BF16, name="osb", bufs=3)
        # one scale piece per engine (Vector/Scalar/GpSimd run in
        # parallel); out-DMA split across the Sync and Scalar HWDGE
        # queues so the two issues and transfers overlap
        nc.vector.tensor_scalar_mul(osb[:, 0:342], po0[:], r[:])
        nc.scalar.activation(osb[:, 342:684], po1[:],
                             mybir.ActivationFunctionType.Copy, scale=r[:])
        nc.vector.tensor_scalar_mul(osb[:, 684:1024], po2[:, 0:340], r[:])
        nc.sync.dma_start(dram["out"][b, bass.ts(ti, P), 0:684],
                          osb[:, 0:684])
        nc.scalar.dma_start(dram["out"][b, bass.ts(ti, P), 684:1024],
                            osb[:, 684:1024])


def _build_nc():
    nc = bass.Bass()
    dram = {
        "qT": nc.declare_dram_parameter("qT", [BPC, E, T], BF16, isOutput=False),
        "kT": nc.declare_dram_parameter("kT", [BPC, E, T], BF16, isOutput=False),
        "vT": nc.declare_dram_parameter("vT", [BPC, E, T], BF16, isOutput=False),
        # "wq" holds W = Wq.T @ Wk (k-proj folded on host)
        "wq": nc.declare_dram_parameter("wq", [E, H], BF16, isOutput=False),
        "wv": nc.declare_dram_parameter("wv", [E, H], BF16, isOutput=False),
        # pad01 pre-laid-out host-side as [P, NC*8]: col c*8+j = chunk-c
        # pad column (replicated 8x for the N=8 den matmul)
        "pad": nc.declare_dram_parameter("pad", [BPC, P, NC * 8], BF16, isOutput=False),
        "out": nc.declare_dram_parameter("out", [BPC, T, H], BF16, isOutput=True),
    }
    with tile.TileContext(nc) as tc, ExitStack() as ctx:
        sb = ctx.enter_context(tc.tile_pool(name="sb", bufs=1))
        ps = ctx.enter_context(tc.tile_pool(name="ps", bufs=8, space="PSUM"))

        pools = {"sb": sb, "ps": ps}
        for wname in ("wq", "wv"):
            pools[wname] = [
                sb.tile([P, H], BF16, name=f"{wname}{ec}") for ec in range(NC)
            ]

        def w_dma(wname, eng):
            def go(ec):
                wt = pools[wname][ec]
                if wname == "wq" and ec == 0:
                    # first W chunk split: pass A reads only cols 0:512
                    eng.dma_start(wt[:, 0:512],
                                  dram[wname][bass.ts(ec, P), 0:512])
                    eng.dma_start(wt[:, 512:1024],
                                  dram[wname][bass.ts(ec, P), 512:1024])
                else:
                    eng.dma_start(wt[:], dram[wname][bass.ts(ec, P), :])
            return go

        # Weight DMAs interleave chunk-by-chunk with batch 0's input loads.
        # wq issues from the idle Scalar HWDGE queue at startup, in parallel
        # with qin on Sync; wv stays on Sync (Scalar is busy by then).
        pools["wq_dma"] = w_dma("wq", nc.scalar)
        pools["wv_dma"] = w_dma("wv", nc.sync)

        # Small PE warm-up starting at sequencer boot: HAM (full clock)
        # trips ~4.5us after sustained PE activity, so early junk matmuls
        # get the clock to 2.4GHz before the first DMA-fed real matmul.
        warm = sb.tile([P, 512], BF16, name="warm")
        nc.gpsimd.memset(warm[:], 0.0)
        wps = ps.tile([P, 512], F32, name="ps")
        for _ in range(9):
            nc.tensor.matmul(wps[:], lhsT=warm[:, 0:P], rhs=warm[:],
                             start=True, stop=True)

        for b in range(BPC):
            _emit_batch(nc, pools, b, dram)

    _split_multi_waits(nc)
    return nc


def _get_nc():
    global _nc_cache
    if _nc_cache is None:
        _nc_cache = _build_nc()
    return _nc_cache


def _make_in_maps(key, query, value, padding_mask, Wk, Wq, Wv):
    bf = ml_dtypes.bfloat16
    # Fold the k-projection into the q side: q @ k.T = query @ W @ key.T
    W = (Wq.astype(np.float64).T @ Wk.astype(np.float64)).astype(np.float32)
    wq = np.ascontiguousarray(W).astype(bf)  # [E, E]
    wv = np.ascontiguousarray(Wv.T).astype(bf)
    pad01 = (padding_mask.reshape(NB, T) == 0).astype(np.float32)  # [B,T]
    in_maps = []
    for c in range(NCORES):
        s = slice(BPC * c, BPC * (c + 1))
        qT = np.ascontiguousarray(query[s].transpose(0, 2, 1)).astype(bf)
        kT = np.ascontiguousarray(key[s].transpose(0, 2, 1)).astype(bf)
        vTf = value[s].transpose(0, 2, 1) * pad01[s][:, None, :]
        vT = np.ascontiguousarray(vTf).astype(bf)
        in_maps.append({
            "qT": qT, "kT": kT, "vT": vT,
            "wq": wq, "wv": wv,
            "pad": np.ascontiguousarray(
                np.repeat(
                    pad01[s].reshape(BPC, NC, P).transpose(0, 2, 1)[..., None],
                    8, axis=3,
                ).reshape(BPC, P, NC * 8)
            ).astype(bf),
        })
    return in_maps


def run_on_cores(in_maps, trace=False, **kw):
    nc = _get_nc()
    return run_bass_kernel_spmd(nc, in_maps, list(range(NCORES)), trace=trace, **kw)


def kernel(key, query, value, padding_mask, Wk, Wq, Wv):
    key = np.asarray(key)
    query = np.asarray(query)
    value = np.asarray(value)
    padding_mask = np.asarray(padding_mask)
    in_maps = _make_in_maps(key, query, value, padding_mask,
                            np.asarray(Wk), np.asarray(Wq), np.asarray(Wv))
    res = run_on_cores(in_maps)
    out = np.empty((NB, T, H), np.float32)
    for c in range(NCORES):
        out[BPC * c: BPC * (c + 1)] = res.results[c]["out"].astype(np.float32)
    return out



# revision 68
# speedup vs baseline: 1.0066x; 1.0059x over previous
"""Causal (diagonal=1) multi-head-of-one attention for trn2, 8-core SPMD.

Reference computation (fp32):
    k = key @ Wk.T; q = query @ Wq.T; v = value @ Wv.T       # [B,T,H]
    qk = (q @ k.T) / sqrt(E)                                  # [B,T,T]
    qk masked with tril(ones, k=1) and padding_mask           # -inf outside
    attn = softmax(qk, -1) @ v                                # [B,T,H]

Sharding: data-parallel over batch, 2 batches per core, no collectives.

The k-projection is folded away on the host: qk = q @ k.T =
query @ (Wq.T @ Wk) @ key.T, so the device multiplies query by the
precomputed W = Wq.T @ Wk and scores directly against the raw key.
This removes 1024^3 MACs per batch (~23% of total PE work).

Device kernel (per core, per batch), all matmuls bf16 with fp32 PSUM:
    tmpT[f,t] = sum_e W[e,f].T-chunks @ queryT[e,t]           (proj)
    v[s,h]    = valueT[e,s].T @ WvT[e,h]
    sT[s,t]   = keyT-chunk.T @ tmpT  (only causally-live s-chunks)
    pT[s,t]   = exp(sT/32)  (ScalarE; max-subtraction skipped: |s/32| <~ 6)
    pT        = affine_select(pT, keep j<=i+1, else 0)        (GPSIMD)
    num[t,h]  = pT-chunk.T @ v ; den[t,1] = pT-chunk.T @ pad01
    out[t,h]  = num * reciprocal(den)                         (VectorE)

Perf notes (205us vs 183us bf16-PE floor on trn2 @2.4GHz):
  - k-proj folded into W on host (-27us/batch of PE work)
  - exact-causal 43-block scores via 512-wide column-trimmed groups
  - batch-0 q-projection runs ec-outer (two passes of 8 PSUM banks) so
    matmuls consume input chunks as their DMAs land
  - DMA issue is serialized ~0.65us/dma_start per issuing engine; weight
    loads issue from the Scalar HWDGE queue in parallel with Sync
  - junk-matmul warmup trips the HAM clock gate (1.2->2.4GHz, ~4.5us
    after first PE activity) before the first data-dependent matmul
  - bf16 output (upcast on host) halves output DMA

padding_mask is folded in exactly on the host: v rows and the denominator
column are scaled by pad01 = (padding_mask == 0), which equals softmax
with -inf at padded keys.
"""
from contextlib import ExitStack

import numpy as np
import ml_dtypes

import concourse.bass as bass
import concourse.mybir as mybir
import concourse.tile as tile
from concourse.bass_utils import run_bass_kernel_spmd

BF16 = mybir.dt.bfloat16
F32 = mybir.dt.float32
P = 128
T = 1024           # sequence length
E = 1024           # embed dim
H = 1024           # head dim
NB = 16            # full batch
NCORES = 8
BPC = NB // NCORES  # batches per core
NC = T // P        # 128-chunks per dim (8)
SCALE = 1.0 / 32.0  # 1/sqrt(E)

_nc_cache = None


# --- walrus workaround: one sync-wait per instruction ---------------------
def _split_multi_waits(nc):
    """This walrus build rejects instructions with >1 sync wait (2 for
    EventSemaphore).  Move extra waits onto fresh same-engine NOPs placed
    immediately before the instruction; per-engine in-order execution
    preserves the gating, and semaphore updates stay on the original."""
    for fn in nc.m.functions:
        for bb in fn.blocks:
            il = bb.instructions
            idx = 0
            while idx < len(il):
                inst = il[idx]
                si = inst.sync_info
                waits = list(si.on_wait) if si and si.on_wait else []
                cap = 2 if isinstance(inst, mybir.InstEventSemaphore) else 1
                if len(waits) > cap:
                    extra, keep = waits[:-cap], waits[-cap:]
                    for j, w in enumerate(extra):
                        nop = mybir.InstNoOp(
                            name=f"I-wsplit-{inst.name}-{j}",
                            engine=inst.engine,
                            ins=[],
                            outs=[],
                            sync_info=mybir.SyncInfo(on_wait=[w], on_update=[]),
                        )
                        il.insert(idx, nop)
                        idx += 1
                    inst.sync_info = mybir.SyncInfo(
                        on_wait=keep, on_update=list(si.on_update or [])
                    )
                idx += 1


def _n_sc(ti):
    """Number of live 128-wide s-chunks for t-tile ti (cols j <= t+1)."""
    return min(ti + 2, NC)


def _emit_batch(nc, pools, b, dram):
    Exp = mybir.ActivationFunctionType.Exp
    w_q, w_v = pools["wq"], pools["wv"]
    sb, ps = pools["sb"], pools["ps"]

    # -- load inputs + projections, ordered so the first projection's DMAs
    #    issue first and later tensors stream in behind the PE --
    # DMA issue is serialized per engine (~0.65us per dma_start DIRECT2D on
    # the issuing sequencer), so weight-chunk DMAs issue from the Scalar
    # HWDGE queue in parallel with input chunks on the Sync queue.
    def load_in(tag, dname, interleave=None, eng=None):
        # interleave: per-ec callback issuing the matching weight-chunk DMA
        # (on the other engine) right after the input chunk, so the ec-th
        # matmul's operands arrive together.
        tiles = []
        for ec in range(NC):
            if interleave is not None:
                interleave(ec)
            t = sb.tile([P, T], BF16, name=f"{tag}{ec}")
            (eng or nc.sync).dma_start(t[:], dram[dname][b, bass.ts(ec, P), :])
            tiles.append(t)
        return tiles

    qTs = [sb.tile([P, T], BF16, name=f"qTs{h}") for h in range(NC)]
    # col 1024 of each v chunk holds the pad01 column, so the softmax
    # denominator rides in the third pv matmul instead of an N=1 matmul
    v_sb = [sb.tile([P, T + 1], BF16, name=f"vsb{s}") for s in range(NC)]

    def proj_qk(w_t, x_in, x_out):
        for ht in range(NC):
            for tg in range(2):
                acc = ps.tile([P, 512], F32, name="ps")
                for ec in range(NC):
                    nc.tensor.matmul(
                        acc[:],
                        lhsT=w_t[ec][:, bass.ts(ht, P)],
                        rhs=x_in[ec][:, bass.ts(tg, 512)],
                        start=(ec == 0),
                        stop=(ec == NC - 1),
                    )
                nc.scalar.copy(x_out[ht][:, bass.ts(tg, 512)], acc[:])

    def proj_ec_outer(w_t, x_in, x_out):
        # ec-outer, two passes of 8 concurrent PSUM accumulations: each
        # ec-step consumes input chunk ec as soon as its DMA lands, so the
        # projection overlaps the initial HBM fill window instead of
        # waiting for all 16 input tiles.  Also self-warms the HAM clock.
        for half in range(2):
            accs = {}
            for ht in range(4 * half, 4 * half + 4):
                for tg in range(2):
                    accs[(ht, tg)] = ps.tile([P, 512], F32, name="ps")
            for ec in range(NC):
                for ht in range(4 * half, 4 * half + 4):
                    for tg in range(2):
                        nc.tensor.matmul(
                            accs[(ht, tg)][:],
                            lhsT=w_t[ec][:, bass.ts(ht, P)],
                            rhs=x_in[ec][:, bass.ts(tg, 512)],
                            start=(ec == 0),
                            stop=(ec == NC - 1),
                        )
            for (ht, tg), acc in accs.items():
                nc.scalar.copy(x_out[ht][:, bass.ts(tg, 512)], acc[:])

    qin = load_in("qin", "qT", interleave=pools.pop("wq_dma", None))
    if b == 0:
        proj_ec_outer(w_q, qin, qTs)
    else:
        proj_qk(w_q, qin, qTs)
    # raw key^T chunks feed the score matmuls directly (k-proj folded into W)
    kTs = load_in("kTs", "kT", interleave=pools.pop("wv_dma", None))
    vin = load_in("vin", "vT")
    padt = sb.tile([P, NC * 8], BF16, name="padt", bufs=2)
    nc.sync.dma_start(padt[:], dram["pad"][b])
    for st in range(NC):
        for hh in range(2):
            acc = ps.tile([P, 512], F32, name="ps")
            for ec in range(NC):
                nc.tensor.matmul(
                    acc[:],
                    lhsT=vin[ec][:, bass.ts(st, P)],
                    rhs=w_v[ec][:, bass.ts(hh, 512)],
                    start=(ec == 0),
                    stop=(ec == NC - 1),
                )
            nc.vector.tensor_copy(v_sb[st][:, bass.ts(hh, 512)], acc[:])
        nc.gpsimd.tensor_copy(v_sb[st][:, T:T + 1], padt[:, st * 8:st * 8 + 1])

    # -- scores^T + exp + causal zeroing --
    # 512-wide t-groups, trimmed to the causally-live column window at
    # 128-col granularity: exactly the causal-minimum 43 block-equivalents
    # in only 13 matmul groups (vs 43 at 128-wide grouping).
    GW = 512
    pT = [sb.tile([P, T], BF16, name=f"pT{s}") for s in range(NC)]
    for g in range(T // GW):
        for sc in range(min((GW * (g + 1)) // P + 1, NC)):
            off = 128 * sc - GW * g
            # live column window: t >= s - 1 first holds at t_local = off-128
            c0 = max(0, off - 128)
            cw = GW - c0
            acc = ps.tile([P, 512], F32, name="ps")
            for hc in range(NC):
                nc.tensor.matmul(
                    acc[:, :cw],
                    lhsT=kTs[hc][:, bass.ts(sc, P)],
                    rhs=qTs[hc][:, GW * g + c0: GW * g + GW],
                    start=(hc == 0),
                    stop=(hc == NC - 1),
                )
            dst = pT[sc][:, GW * g + c0: GW * g + GW]
            nc.scalar.activation(dst, acc[:, :cw], Exp, scale=SCALE)
            if off >= 0:
                # keep where t_local - s_local - (off-c0) + 1 >= 0 (j <= i+1)
                nc.gpsimd.affine_select(
                    out=dst,
                    in_=dst,
                    compare_op=mybir.AluOpType.is_ge,
                    fill=0.0,
                    base=1 - (off - c0),
                    pattern=[[1, cw]],
                    channel_multiplier=-1,
                )

    # -- attn = (pT.T @ [v | pad01]) with post-normalization --
    # three ~342-col matmuls per s-chunk (1025 moving cycles total, same
    # as 512+512+1) keep every matmul wide enough to pipeline its
    # ldweights; the denominator is column 340 of po2.
    # Ascending tile order: ending on the big ti=7 tile (3.4us of PE)
    # lets every prior tile's scale+DMA chain drain before the kernel
    # tail, which beats reordering small tiles last (two pending chains
    # at the end cost more than the ~60ns mid-phase bank-recycle stalls
    # ascending incurs at ti=2,3).
    for ti in range(NC):
        nsc = _n_sc(ti)
        po0 = ps.tile([P, 342], F32, name="ps")
        po1 = ps.tile([P, 342], F32, name="ps")
        po2 = ps.tile([P, 341], F32, name="ps")
        for sc in range(nsc):
            lhsT = pT[sc][:, bass.ts(ti, P)]
            st, sp = (sc == 0), (sc == nsc - 1)
            # po2 (carrying the denominator) first, so the reciprocal and
            # its dependent scale overlap the last two matmuls of the tile
            nc.tensor.matmul(po2[:], lhsT=lhsT, rhs=v_sb[sc][:, 684:1025],
                             start=st, stop=sp)
            nc.tensor.matmul(po0[:], lhsT=lhsT, rhs=v_sb[sc][:, 0:342],
                             start=st, stop=sp)
            nc.tensor.matmul(po1[:], lhsT=lhsT, rhs=v_sb[sc][:, 342:684],
                             start=st, stop=sp)
        r = sb.tile([P, 1], F32, name="recip", bufs=3)
        nc.vector.reciprocal(r[:], po2[:, 340:341])
        osb = sb.tile([P, T], BF16, name="osb", bufs=3)
        # one scale piece per engine (Vector/Scalar/GpSimd run in
        # parallel); out-DMA split across the Sync and Scalar HWDGE
        # queues so the two issues and transfers overlap
        nc.vector.tensor_scalar_mul(osb[:, 0:342], po0[:], r[:])
        nc.scalar.activation(osb[:, 342:684], po1[:],
                             mybir.ActivationFunctionType.Copy, scale=r[:])
        nc.vector.tensor_scalar_mul(osb[:, 684:1024], po2[:, 0:340], r[:])
        nc.sync.dma_start(dram["out"][b, bass.ts(ti, P), 0:684],
                          osb[:, 0:684])
        nc.scalar.dma_start(dram["out"][b, bass.ts(ti, P), 684:1024],
                            osb[:, 684:1024])


def _build_nc():
    nc = bass.Bass()
    dram = {
        "qT": nc.declare_dram_parameter("qT", [BPC, E, T], BF16, isOutput=False),
        "kT": nc.declare_dram_parameter("kT", [BPC, E, T], BF16, isOutput=False),
        "vT": nc.declare_dram_parameter("vT", [BPC, E, T], BF16, isOutput=False),
        # "wq" holds W = Wq.T @ Wk (k-proj folded on host)
        "wq": nc.declare_dram_parameter("wq", [E, H], BF16, isOutput=False),
        "wv": nc.declare_dram_parameter("wv", [E, H], BF16, isOutput=False),
        # pad01 pre-laid-out host-side as [P, NC*8]: col c*8+j = chunk-c
        # pad column (replicated 8x for the N=8 den matmul)
        "pad": nc.declare_dram_parameter("pad", [BPC, P, NC * 8], BF16, isOutput=False),
        "out": nc.declare_dram_parameter("out", [BPC, T, H], BF16, isOutput=True),
    }
    with tile.TileContext(nc) as tc, ExitStack() as ctx:
        sb = ctx.enter_context(tc.tile_pool(name="sb", bufs=1))
        ps = ctx.enter_context(tc.tile_pool(name="ps", bufs=8, space="PSUM"))

        pools = {"sb": sb, "ps": ps}
        for wname in ("wq", "wv"):
            pools[wname] = [
                sb.tile([P, H], BF16, name=f"{wname}{ec}") for ec in range(NC)
            ]

        def w_dma(wname, eng):
            def go(ec):
                eng.dma_start(
                    pools[wname][ec][:], dram[wname][bass.ts(ec, P), :]
                )
            return go

        # Weight DMAs interleave chunk-by-chunk with batch 0's input loads.
        # wq issues from the idle Scalar HWDGE queue at startup, in parallel
        # with qin on Sync; wv stays on Sync (Scalar is busy by then).
        pools["wq_dma"] = w_dma("wq", nc.scalar)
        pools["wv_dma"] = w_dma("wv", nc.sync)

        # Small PE warm-up starting at sequencer boot: HAM (full clock)
        # trips ~4.5us after sustained PE activity, so early junk matmuls
        # get the clock to 2.4GHz before the first DMA-fed real matmul.
        warm = sb.tile([P, 512], BF16, name="warm")
        nc.gpsimd.memset(warm[:], 0.0)
        wps = ps.tile([P, 512], F32, name="ps")
        for _ in range(11):
            nc.tensor.matmul(wps[:], lhsT=warm[:, 0:P], rhs=warm[:],
                             start=True, stop=True)

        for b in range(BPC):
            _emit_batch(nc, pools, b, dram)

    _split_multi_waits(nc)
    return nc


def _get_nc():
    global _nc_cache
    if _nc_cache is None:
        _nc_cache = _build_nc()
    return _nc_cache


def _make_in_maps(key, query, value, padding_mask, Wk, Wq, Wv):
    bf = ml_dtypes.bfloat16
    # Fold the k-projection into the q side: q @ k.T = query @ W @ key.T
    W = (Wq.astype(np.float64).T @ Wk.astype(np.float64)).astype(np.float32)
    wq = np.ascontiguousarray(W).astype(bf)  # [E, E]
    wv = np.ascontiguousarray(Wv.T).astype(bf)
    pad01 = (padding_mask.reshape(NB, T) == 0).astype(np.float32)  # [B,T]
    in_maps = []
    for c in range(NCORES):
        s = slice(BPC * c, BPC * (c + 1))
        qT = np.ascontiguousarray(query[s].transpose(0, 2, 1)).astype(bf)
        kT = np.ascontiguousarray(key[s].transpose(0, 2, 1)).astype(bf)
        vTf = value[s].transpose(0, 2, 1) * pad01[s][:, None, :]
        vT = np.ascontiguousarray(vTf).astype(bf)
        in_maps.append({
            "qT": qT, "kT": kT, "vT": vT,
            "wq": wq, "wv": wv,
            "pad": np.ascontiguousarray(
                np.repeat(
                    pad01[s].reshape(BPC, NC, P).transpose(0, 2, 1)[..., None],
                    8, axis=3,
                ).reshape(BPC, P, NC * 8)
            ).astype(bf),
        })
    return in_maps


def run_on_cores(in_maps, trace=False, **kw):
    nc = _get_nc()
    return run_bass_kernel_spmd(nc, in_maps, list(range(NCORES)), trace=trace, **kw)


def kernel(key, query, value, padding_mask, Wk, Wq, Wv):
    key = np.asarray(key)
    query = np.asarray(query)
    value = np.asarray(value)
    padding_mask = np.asarray(padding_mask)
    in_maps = _make_in_maps(key, query, value, padding_mask,
                            np.asarray(Wk), np.asarray(Wq), np.asarray(Wv))
    res = run_on_cores(in_maps)
    out = np.empty((NB, T, H), np.float32)
    for c in range(NCORES):
        out[BPC * c: BPC * (c + 1)] = res.results[c]["out"].astype(np.float32)
    return out



# revision 69
# speedup vs baseline: 1.0081x; 1.0014x over previous
"""Causal (diagonal=1) multi-head-of-one attention for trn2, 8-core SPMD.

Reference computation (fp32):
    k = key @ Wk.T; q = query @ Wq.T; v = value @ Wv.T       # [B,T,H]
    qk = (q @ k.T) / sqrt(E)                                  # [B,T,T]
    qk masked with tril(ones, k=1) and padding_mask           # -inf outside
    attn = softmax(qk, -1) @ v                                # [B,T,H]

Sharding: data-parallel over batch, 2 batches per core, no collectives.

The k-projection is folded away on the host: qk = q @ k.T =
query @ (Wq.T @ Wk) @ key.T, so the device multiplies query by the
precomputed W = Wq.T @ Wk and scores directly against the raw key.
This removes 1024^3 MACs per batch (~23% of total PE work).

Device kernel (per core, per batch), all matmuls bf16 with fp32 PSUM:
    tmpT[f,t] = sum_e W[e,f].T-chunks @ queryT[e,t]           (proj)
    v[s,h]    = valueT[e,s].T @ WvT[e,h]
    sT[s,t]   = keyT-chunk.T @ tmpT  (only causally-live s-chunks)
    pT[s,t]   = exp(sT/32)  (ScalarE; max-subtraction skipped: |s/32| <~ 6)
    pT        = affine_select(pT, keep j<=i+1, else 0)        (GPSIMD)
    num[t,h]  = pT-chunk.T @ v ; den[t,1] = pT-chunk.T @ pad01
    out[t,h]  = num * reciprocal(den)                         (VectorE)

Perf notes (205us vs 183us bf16-PE floor on trn2 @2.4GHz):
  - k-proj folded into W on host (-27us/batch of PE work)
  - exact-causal 43-block scores via 512-wide column-trimmed groups
  - batch-0 q-projection runs ec-outer (two passes of 8 PSUM banks) so
    matmuls consume input chunks as their DMAs land
  - DMA issue is serialized ~0.65us/dma_start per issuing engine; weight
    loads issue from the Scalar HWDGE queue in parallel with Sync
  - junk-matmul warmup trips the HAM clock gate (1.2->2.4GHz, ~4.5us
    after first PE activity) before the first data-dependent matmul
  - bf16 output (upcast on host) halves output DMA

padding_mask is folded in exactly on the host: v rows and the denominator
column are scaled by pad01 = (padding_mask == 0), which equals softmax
with -inf at padded keys.
"""
import os
# reset cores at runtime init: protects against a previously-wedged device
# (NRT_EXEC_UNIT_UNRECOVERABLE observed once in testing); must be set
# before the neuron runtime initializes
os.environ.setdefault("NEURON_RT_RESET_CORES", "1")

from contextlib import ExitStack

import numpy as np
import ml_dtypes

import concourse.bass as bass
import concourse.mybir as mybir
import concourse.tile as tile
from concourse.bass_utils import run_bass_kernel_spmd

BF16 = mybir.dt.bfloat16
F32 = mybir.dt.float32
P = 128
T = 1024           # sequence length
E = 1024           # embed dim
H = 1024           # head dim
NB = 16            # full batch
NCORES = 8
BPC = NB // NCORES  # batches per core
NC = T // P        # 128-chunks per dim (8)
SCALE = 1.0 / 32.0  # 1/sqrt(E)

_nc_cache = None


# --- walrus workaround: one sync-wait per instruction ---------------------
def _split_multi_waits(nc):
    """This walrus build rejects instructions with >1 sync wait (2 for
    EventSemaphore).  Move extra waits onto fresh same-engine NOPs placed
    immediately before the instruction; per-engine in-order execution
    preserves the gating, and semaphore updates stay on the original."""
    for fn in nc.m.functions:
        for bb in fn.blocks:
            il = bb.instructions
            idx = 0
            while idx < len(il):
                inst = il[idx]
                si = inst.sync_info
                waits = list(si.on_wait) if si and si.on_wait else []
                cap = 2 if isinstance(inst, mybir.InstEventSemaphore) else 1
                if len(waits) > cap:
                    extra, keep = waits[:-cap], waits[-cap:]
                    for j, w in enumerate(extra):
                        nop = mybir.InstNoOp(
                            name=f"I-wsplit-{inst.name}-{j}",
                            engine=inst.engine,
                            ins=[],
                            outs=[],
                            sync_info=mybir.SyncInfo(on_wait=[w], on_update=[]),
                        )
                        il.insert(idx, nop)
                        idx += 1
                    inst.sync_info = mybir.SyncInfo(
                        on_wait=keep, on_update=list(si.on_update or [])
                    )
                idx += 1


def _n_sc(ti):
    """Number of live 128-wide s-chunks for t-tile ti (cols j <= t+1)."""
    return min(ti + 2, NC)


def _emit_batch(nc, pools, b, dram):
    Exp = mybir.ActivationFunctionType.Exp
    w_q, w_v = pools["wq"], pools["wv"]
    sb, ps = pools["sb"], pools["ps"]

    # -- load inputs + projections, ordered so the first projection's DMAs
    #    issue first and later tensors stream in behind the PE --
    # DMA issue is serialized per engine (~0.65us per dma_start DIRECT2D on
    # the issuing sequencer), so weight-chunk DMAs issue from the Scalar
    # HWDGE queue in parallel with input chunks on the Sync queue.
    def load_in(tag, dname, interleave=None, eng=None):
        # interleave: per-ec callback issuing the matching weight-chunk DMA
        # (on the other engine) right after the input chunk, so the ec-th
        # matmul's operands arrive together.
        tiles = []
        for ec in range(NC):
            if interleave is not None:
                interleave(ec)
            t = sb.tile([P, T], BF16, name=f"{tag}{ec}")
            (eng or nc.sync).dma_start(t[:], dram[dname][b, bass.ts(ec, P), :])
            tiles.append(t)
        return tiles

    qTs = [sb.tile([P, T], BF16, name=f"qTs{h}") for h in range(NC)]
    # col 1024 of each v chunk holds the pad01 column, so the softmax
    # denominator rides in the third pv matmul instead of an N=1 matmul
    v_sb = [sb.tile([P, T + 1], BF16, name=f"vsb{s}") for s in range(NC)]

    def proj_qk(w_t, x_in, x_out):
        for ht in range(NC):
            for tg in range(2):
                acc = ps.tile([P, 512], F32, name="ps")
                for ec in range(NC):
                    nc.tensor.matmul(
                        acc[:],
                        lhsT=w_t[ec][:, bass.ts(ht, P)],
                        rhs=x_in[ec][:, bass.ts(tg, 512)],
                        start=(ec == 0),
                        stop=(ec == NC - 1),
                    )
                nc.scalar.copy(x_out[ht][:, bass.ts(tg, 512)], acc[:])

    def proj_ec_outer(w_t, x_in, x_out):
        # ec-outer, two passes of 8 concurrent PSUM accumulations: each
        # ec-step consumes input chunk ec as soon as its DMA lands, so the
        # projection overlaps the initial HBM fill window instead of
        # waiting for all 16 input tiles.  Also self-warms the HAM clock.
        for half in range(2):
            accs = {}
            for ht in range(4 * half, 4 * half + 4):
                for tg in range(2):
                    accs[(ht, tg)] = ps.tile([P, 512], F32, name="ps")
            for ec in range(NC):
                for ht in range(4 * half, 4 * half + 4):
                    for tg in range(2):
                        nc.tensor.matmul(
                            accs[(ht, tg)][:],
                            lhsT=w_t[ec][:, bass.ts(ht, P)],
                            rhs=x_in[ec][:, bass.ts(tg, 512)],
                            start=(ec == 0),
                            stop=(ec == NC - 1),
                        )
            for (ht, tg), acc in accs.items():
                nc.scalar.copy(x_out[ht][:, bass.ts(tg, 512)], acc[:])

    qin = load_in("qin", "qT", interleave=pools.pop("wq_dma", None))
    if b == 0:
        proj_ec_outer(w_q, qin, qTs)
    else:
        proj_qk(w_q, qin, qTs)
    # raw key^T chunks feed the score matmuls directly (k-proj folded into W)
    kTs = load_in("kTs", "kT", interleave=pools.pop("wv_dma", None))
    vin = load_in("vin", "vT")
    padt = sb.tile([P, NC * 8], BF16, name="padt", bufs=2)
    nc.sync.dma_start(padt[:], dram["pad"][b])
    for st in range(NC):
        for hh in range(2):
            acc = ps.tile([P, 512], F32, name="ps")
            for ec in range(NC):
                nc.tensor.matmul(
                    acc[:],
                    lhsT=vin[ec][:, bass.ts(st, P)],
                    rhs=w_v[ec][:, bass.ts(hh, 512)],
                    start=(ec == 0),
                    stop=(ec == NC - 1),
                )
            nc.vector.tensor_copy(v_sb[st][:, bass.ts(hh, 512)], acc[:])
        nc.gpsimd.tensor_copy(v_sb[st][:, T:T + 1], padt[:, st * 8:st * 8 + 1])

    # -- scores^T + exp + causal zeroing --
    # 512-wide t-groups, trimmed to the causally-live column window at
    # 128-col granularity: exactly the causal-minimum 43 block-equivalents
    # in only 13 matmul groups (vs 43 at 128-wide grouping).
    GW = 512
    pT = [sb.tile([P, T], BF16, name=f"pT{s}") for s in range(NC)]
    for g in range(T // GW):
        for sc in range(min((GW * (g + 1)) // P + 1, NC)):
            off = 128 * sc - GW * g
            # live column window: t >= s - 1 first holds at t_local = off-128
            c0 = max(0, off - 128)
            cw = GW - c0
            acc = ps.tile([P, 512], F32, name="ps")
            for hc in range(NC):
                nc.tensor.matmul(
                    acc[:, :cw],
                    lhsT=kTs[hc][:, bass.ts(sc, P)],
                    rhs=qTs[hc][:, GW * g + c0: GW * g + GW],
                    start=(hc == 0),
                    stop=(hc == NC - 1),
                )
            dst = pT[sc][:, GW * g + c0: GW * g + GW]
            nc.scalar.activation(dst, acc[:, :cw], Exp, scale=SCALE)
            if off >= 0:
                # keep where t_local - s_local - (off-c0) + 1 >= 0 (j <= i+1)
                nc.gpsimd.affine_select(
                    out=dst,
                    in_=dst,
                    compare_op=mybir.AluOpType.is_ge,
                    fill=0.0,
                    base=1 - (off - c0),
                    pattern=[[1, cw]],
                    channel_multiplier=-1,
                )

    # -- attn = (pT.T @ [v | pad01]) with post-normalization --
    # three ~342-col matmuls per s-chunk (1025 moving cycles total, same
    # as 512+512+1) keep every matmul wide enough to pipeline its
    # ldweights; the denominator is column 340 of po2.
    # Ascending tile order: ending on the big ti=7 tile (3.4us of PE)
    # lets every prior tile's scale+DMA chain drain before the kernel
    # tail, which beats reordering small tiles last (two pending chains
    # at the end cost more than the ~60ns mid-phase bank-recycle stalls
    # ascending incurs at ti=2,3).
    for ti in range(NC):
        nsc = _n_sc(ti)
        po0 = ps.tile([P, 342], F32, name="ps")
        po1 = ps.tile([P, 342], F32, name="ps")
        po2 = ps.tile([P, 341], F32, name="ps")
        for sc in range(nsc):
            lhsT = pT[sc][:, bass.ts(ti, P)]
            st, sp = (sc == 0), (sc == nsc - 1)
            # po2 (carrying the denominator) first, so the reciprocal and
            # its dependent scale overlap the last two matmuls of the tile
            nc.tensor.matmul(po2[:], lhsT=lhsT, rhs=v_sb[sc][:, 684:1025],
                             start=st, stop=sp)
            nc.tensor.matmul(po0[:], lhsT=lhsT, rhs=v_sb[sc][:, 0:342],
                             start=st, stop=sp)
            nc.tensor.matmul(po1[:], lhsT=lhsT, rhs=v_sb[sc][:, 342:684],
                             start=st, stop=sp)
        r = sb.tile([P, 1], F32, name="recip", bufs=3)
        nc.vector.reciprocal(r[:], po2[:, 340:341])
        osb = sb.tile([P, T], BF16, name="osb", bufs=3)
        # one scale piece per engine (Vector/Scalar/GpSimd run in
        # parallel); out-DMA split across the Sync and Scalar HWDGE
        # queues so the two issues and transfers overlap
        nc.vector.tensor_scalar_mul(osb[:, 0:342], po0[:], r[:])
        nc.scalar.activation(osb[:, 342:684], po1[:],
                             mybir.ActivationFunctionType.Copy, scale=r[:])
        nc.vector.tensor_scalar_mul(osb[:, 684:1024], po2[:, 0:340], r[:])
        nc.sync.dma_start(dram["out"][b, bass.ts(ti, P), 0:684],
                          osb[:, 0:684])
        nc.scalar.dma_start(dram["out"][b, bass.ts(ti, P), 684:1024],
                            osb[:, 684:1024])


def _build_nc():
    nc = bass.Bass()
    dram = {
        "qT": nc.declare_dram_parameter("qT", [BPC, E, T], BF16, isOutput=False),
        "kT": nc.declare_dram_parameter("kT", [BPC, E, T], BF16, isOutput=False),
        "vT": nc.declare_dram_parameter("vT", [BPC, E, T], BF16, isOutput=False),
        # "wq" holds W = Wq.T @ Wk (k-proj folded on host)
        "wq": nc.declare_dram_parameter("wq", [E, H], BF16, isOutput=False),
        "wv": nc.declare_dram_parameter("wv", [E, H], BF16, isOutput=False),
        # pad01 pre-laid-out host-side as [P, NC*8]: col c*8+j = chunk-c
        # pad column (replicated 8x for the N=8 den matmul)
        "pad": nc.declare_dram_parameter("pad", [BPC, P, NC * 8], BF16, isOutput=False),
        "out": nc.declare_dram_parameter("out", [BPC, T, H], BF16, isOutput=True),
    }
    with tile.TileContext(nc) as tc, ExitStack() as ctx:
        sb = ctx.enter_context(tc.tile_pool(name="sb", bufs=1))
        ps = ctx.enter_context(tc.tile_pool(name="ps", bufs=8, space="PSUM"))

        pools = {"sb": sb, "ps": ps}
        for wname in ("wq", "wv"):
            pools[wname] = [
                sb.tile([P, H], BF16, name=f"{wname}{ec}") for ec in range(NC)
            ]

        def w_dma(wname, eng):
            def go(ec):
                eng.dma_start(
                    pools[wname][ec][:], dram[wname][bass.ts(ec, P), :]
                )
            return go

        # Weight DMAs interleave chunk-by-chunk with batch 0's input loads.
        # wq issues from the idle Scalar HWDGE queue at startup, in parallel
        # with qin on Sync; wv stays on Sync (Scalar is busy by then).
        pools["wq_dma"] = w_dma("wq", nc.scalar)
        pools["wv_dma"] = w_dma("wv", nc.sync)

        # Small PE warm-up starting at sequencer boot: HAM (full clock)
        # trips ~4.5us after sustained PE activity, so early junk matmuls
        # get the clock to 2.4GHz before the first DMA-fed real matmul.
        warm = sb.tile([P, 512], BF16, name="warm")
        nc.gpsimd.memset(warm[:], 0.0)
        wps = ps.tile([P, 512], F32, name="ps")
        for _ in range(11):
            nc.tensor.matmul(wps[:], lhsT=warm[:, 0:P], rhs=warm[:],
                             start=True, stop=True)

        for b in range(BPC):
            _emit_batch(nc, pools, b, dram)

    _split_multi_waits(nc)
    return nc


def _get_nc():
    global _nc_cache
    if _nc_cache is None:
        _nc_cache = _build_nc()
    return _nc_cache


def _make_in_maps(key, query, value, padding_mask, Wk, Wq, Wv):
    bf = ml_dtypes.bfloat16
    # Fold the k-projection into the q side: q @ k.T = query @ W @ key.T
    W = (Wq.astype(np.float64).T @ Wk.astype(np.float64)).astype(np.float32)
    wq = np.ascontiguousarray(W).astype(bf)  # [E, E]
    wv = np.ascontiguousarray(Wv.T).astype(bf)
    pad01 = (padding_mask.reshape(NB, T) == 0).astype(np.float32)  # [B,T]
    in_maps = []
    for c in range(NCORES):
        s = slice(BPC * c, BPC * (c + 1))
        qT = np.ascontiguousarray(query[s].transpose(0, 2, 1)).astype(bf)
        kT = np.ascontiguousarray(key[s].transpose(0, 2, 1)).astype(bf)
        vTf = value[s].transpose(0, 2, 1) * pad01[s][:, None, :]
        vT = np.ascontiguousarray(vTf).astype(bf)
        in_maps.append({
            "qT": qT, "kT": kT, "vT": vT,
            "wq": wq, "wv": wv,
            "pad": np.ascontiguousarray(
                np.repeat(
                    pad01[s].reshape(BPC, NC, P).transpose(0, 2, 1)[..., None],
                    8, axis=3,
                ).reshape(BPC, P, NC * 8)
            ).astype(bf),
        })
    return in_maps


def run_on_cores(in_maps, trace=False, **kw):
    nc = _get_nc()
    return run_bass_kernel_spmd(nc, in_maps, list(range(NCORES)), trace=trace, **kw)


def kernel(key, query, value, padding_mask, Wk, Wq, Wv):
    key = np.asarray(key)
    query = np.asarray(query)
    value = np.asarray(value)
    padding_mask = np.asarray(padding_mask)
    in_maps = _make_in_maps(key, query, value, padding_mask,
                            np.asarray(Wk), np.asarray(Wq), np.asarray(Wv))
    res = run_on_cores(in_maps)
    out = np.empty((NB, T, H), np.float32)
    for c in range(NCORES):
        out[BPC * c: BPC * (c + 1)] = res.results[c]["out"].astype(np.float32)
    return out

